# revision 17
# baseline (speedup 1.0000x reference)
"""Channel-permutation (stable bucket sort by cluster id) kernel for TRN2.

out[b, l, c] = x[b, l, order[c]]  with  order = stable argsort(y_pred)

Strategy (8 NeuronCores, data parallel over batch):
  - Each core gets 4 of the 32 batches -> a [16384, 512] fp32 slab.
  - The channel permutation `order` is computed on the host (y_pred is tiny)
    and baked into the program as "runs" (maximal stretches where
    order[c+1] == order[c]+1).  Random 8-cluster y_pred gives ~450 runs,
    pair-merged into ~230 copy instructions via an extra AP dim.
  - Per core the slab is processed in 8 tiles of [128 part x 16 rows x 512 ch]
    (4 MiB): contiguous DMA load (SP/HWDGE), on-chip gather (DVE strided
    copies), contiguous DMA store (ACT/HWDGE).  Double buffered; the gather
    hides under the ~187us/core DMA roofline.

Raw Bass (not Tile): the Tile framework inlines semaphore waits into
instructions, and the hardware allows only 1 inline wait on a DMA and 2 on a
TensorCopy -- the slot-reuse wait sets here exceed that.  With explicit
standalone wait_ge instructions there is no such limit.
"""

import functools
import os
from contextlib import ExitStack

import numpy as np

import concourse.bass as bass
import concourse.mybir as mybir
from concourse.ap import AP
from concourse import bass_utils

N_CORES = 8
B, L, C = 32, 4096, 512
B_PER_CORE = B // N_CORES          # 4
ROWS = B_PER_CORE * L              # 16384 rows per core
P = 128                            # SBUF partitions
R_LO = int(os.environ.get("K_RLO", "24"))  # rows per partition per tile
# "quad" (parallelogram-merged singles, 4-dim APs with two pair dims) is
# ~5% faster but intermittently leaves the device unrecoverable at scale;
# "pair" (max one pair dim per AP) is stable.  Default: stable.
K_PLAN = os.environ.get("K_PLAN", "pair")  # quad | pair
TILE_ROWS = P * R_LO
F32 = mybir.dt.float32


def _runs_from_order(order, c=C):
    """Maximal runs (dst_start, src_start, length) with order[d+i] == s+i."""
    runs = []
    start = 0
    for i in range(1, c + 1):
        if i == c or order[i] != order[i - 1] + 1:
            runs.append((start, int(order[start]), i - start))
            start = i
    return runs


def _plan_jobs(runs):
    """Merge runs into copy jobs, minimizing instruction count.

    A job is (d0, s0, length, extra) where extra is a tuple of up to two
    (dst_step, src_step) dims of count 2.  The AP is
    [partition] + [(step, 2) per extra] + [rows] + ([1, length] if length>1).
    The engine AP limit is 4 dims total, so:
      - length==1 runs: up to 2 extra dims -> merge FOUR runs per
        instruction when they form a parallelogram in (dst, src) space
        (two run-pairs with the same difference vector).
      - length>=2 runs: 1 extra dim -> merge pairs of equal-length runs.
    """
    from collections import defaultdict

    jobs = []
    singles = [r for r in runs if r[2] == 1]
    longs = [r for r in runs if r[2] > 1]
    if K_PLAN == "pair":
        singles, longs = [], runs

    # --- quad-match length-1 runs (parallelogram matching) ---
    n = len(singles)
    buckets = defaultdict(list)
    for i in range(n):
        di, si, _ = singles[i]
        for j in range(i + 1, n):
            dj, sj, _ = singles[j]
            buckets[(dj - di, sj - si)].append((i, j))
    used = [False] * n
    for vec, plist in sorted(buckets.items(), key=lambda kv: -len(kv[1])):
        if len(plist) < 2:
            continue
        chosen, taken = [], set()
        for i, j in plist:
            if used[i] or used[j] or i in taken or j in taken:
                continue
            chosen.append((i, j))
            taken.update((i, j))
        while len(chosen) >= 2:
            i, j = chosen.pop(0)
            k, l = chosen.pop(0)
            for idx in (i, j, k, l):
                used[idx] = True
            d0, s0, _ = singles[i]
            outer = (singles[k][0] - d0, singles[k][1] - s0)
            inner = vec
            jobs.append((d0, s0, 1, (outer, inner)))

    # --- pair leftovers (and length>=2 runs), one extra dim ---
    def pair_up(rs):
        rs = sorted(rs)
        for a in range(0, len(rs) - 1, 2):
            d0, s0, length = rs[a]
            d1, s1, _ = rs[a + 1]
            jobs.append((d0, s0, length, ((d1 - d0, s1 - s0),)))
        if len(rs) % 2:
            d0, s0, length = rs[-1]
            jobs.append((d0, s0, length, ()))

    pair_up([singles[i] for i in range(n) if not used[i]])
    by_len = defaultdict(list)
    for r in longs:
        by_len[r[2]].append(r)
    for _, rs in sorted(by_len.items()):
        pair_up(rs)
    return jobs


def _job_ap(tile_ap, job, use_dst):
    """AP for a job over an SBUF tile viewed as [128, R, C]."""
    d0, s0, length, extra = job
    o0 = d0 if use_dst else s0
    base = tile_ap[:, :, o0:o0 + length]
    dims = [base.ap[0]]
    for dd, ds in extra:
        dims.append([dd if use_dst else ds, 2])
    dims.append(base.ap[1])
    if length > 1:
        dims.append(base.ap[2])
    if not extra and length > 1:
        return base
    return AP(tensor=base.tensor, offset=base.offset, ap=dims)


def _emit(nc, x_ap, out_ap, order, rows, r_lo=R_LO):
    """Emit the per-core program: rows x 512 slab, gather channels by order."""
    jobs = _plan_jobs(_runs_from_order(order))
    rows_pp = rows // P  # rows per partition overall
    # Small first tile -> short pipeline fill; small last tile -> short
    # drain (tail = last gather + last store happens after the final load).
    if rows_pp > 2 * r_lo:
        mid = rows_pp - 16
        tile_rs = [8] + [r_lo] * (mid // r_lo)
        if mid % r_lo:
            tile_rs.append(mid % r_lo)
        tile_rs.append(8)
    else:
        tile_rs = [r_lo] * (rows_pp // r_lo)
        if rows_pp % r_lo:
            tile_rs.append(rows_pp % r_lo)
    assert sum(tile_rs) == rows_pp
    n_tiles = len(tile_rs)
    free = r_lo * C

    # tile t covers rows [off*P, off*P + P*r), partition p owns r of them
    offs = [sum(tile_rs[:t]) for t in range(n_tiles)]

    def x_tile(t):
        r = tile_rs[t]
        return x_ap[offs[t] * P:(offs[t] + r) * P, :].rearrange(
            "(p r) c -> p (r c)", p=P
        )

    def out_tile(t):
        r = tile_rs[t]
        return out_ap[offs[t] * P:(offs[t] + r) * P, :].rearrange(
            "(p r) c -> p (r c)", p=P
        )

    with ExitStack() as ctx:
        in_bufs = [
            ctx.enter_context(nc.sbuf_tensor(f"t_in{i}", [P, free], F32))
            for i in range(2)
        ]
        out_bufs = [
            ctx.enter_context(nc.sbuf_tensor(f"t_out{i}", [P, free], F32))
            for i in range(2)
        ]
        # Per-parity DMA sems: at most one DMA in flight per sem, so a
        # sem value unambiguously identifies WHICH transfer completed
        # (same-ring DMA completions are not ordered).
        s_load = [
            ctx.enter_context(nc.semaphore(f"s_load{i}")) for i in range(2)
        ]
        s_store = [
            ctx.enter_context(nc.semaphore(f"s_store{i}")) for i in range(2)
        ]
        s_dve = ctx.enter_context(nc.semaphore("s_dve"))
        ctx.enter_context(nc.Block())
        block = nc.cur_block

        @block.sync
        def _(sync):
            for t in range(n_tiles):
                if t >= 2:
                    # in_bufs[t%2] is free once tile t-2's copies retired
                    sync.wait_ge(s_dve, t - 1)
                r = tile_rs[t]
                sync.dma_start(
                    in_bufs[t % 2][:, :r * C], x_tile(t)
                ).then_inc(s_load[t % 2], 16)

        @block.scalar
        def _(scalar):
            for t in range(n_tiles):
                scalar.wait_ge(s_dve, t + 1)  # tile t fully gathered
                r = tile_rs[t]
                scalar.dma_start(
                    out_tile(t), out_bufs[t % 2][:, :r * C]
                ).then_inc(s_store[t % 2], 16)
            # Drain: the program must not end with store transfers still in
            # flight (teardown while a DMA is outstanding intermittently
            # kills the exec unit).  DVE only ever waits stores up to t-2,
            # so explicitly await every store completion here.
            for par in range(2):
                n_par = len([t for t in range(n_tiles) if t % 2 == par])
                if n_par:
                    scalar.wait_ge(s_store[par], 16 * n_par)

        @block.vector
        def _(vector):
            for t in range(n_tiles):
                vector.wait_ge(s_load[t % 2], 16 * (t // 2 + 1))  # load t done
                if t >= 2:
                    # out_bufs[t%2] is free once store t-2 drained
                    vector.wait_ge(s_store[t % 2], 16 * ((t - 2) // 2 + 1))
                r = tile_rs[t]
                src_t = in_bufs[t % 2][:, :r * C].rearrange(
                    "p (r c) -> p r c", c=C
                )
                dst_t = out_bufs[t % 2][:, :r * C].rearrange(
                    "p (r c) -> p r c", c=C
                )
                for i, job in enumerate(jobs):
                    ins = vector.tensor_copy(
                        out=_job_ap(dst_t, job, True),
                        in_=_job_ap(src_t, job, False),
                    )
                    if i == len(jobs) - 1:
                        ins.then_inc(s_dve, 1)


@functools.lru_cache(maxsize=4)
def _build(order_key):
    nc = bass.Bass("TRN2")
    x = nc.dram_tensor("x", [ROWS, C], F32, kind="ExternalInput")
    out = nc.dram_tensor("out", [ROWS, C], F32, kind="ExternalOutput")
    _emit(nc, x[:], out[:], list(order_key), ROWS)
    return nc


def _run(x, y_pred, trace=False, trace_cores=None):
    x = np.ascontiguousarray(np.asarray(x), dtype=np.float32)
    y_pred = np.asarray(y_pred)
    assert x.shape == (B, L, C), x.shape
    order = np.argsort(y_pred, kind="stable")
    nc = _build(tuple(int(v) for v in order))

    shards = [
        np.ascontiguousarray(
            x[i * B_PER_CORE:(i + 1) * B_PER_CORE].reshape(ROWS, C)
        )
        for i in range(N_CORES)
    ]
    in_maps = [{"x": s} for s in shards]
    res = bass_utils.run_bass_kernel_spmd(
        nc,
        in_maps,
        core_ids=list(range(N_CORES)),
        trace=trace,
        trace_cores=trace_cores,
    )
    out = np.concatenate(
        [r["out"].reshape(B_PER_CORE, L, C) for r in res.results], axis=0
    )
    return out, res


def kernel(x, y_pred):
    out, _ = _run(x, y_pred, trace=False)
    return out


# revision 19
# speedup vs baseline: 1.1085x; 1.1085x over previous
"""Channel-permutation (stable bucket sort by cluster id) kernel for TRN2.

out[b, l, c] = x[b, l, order[c]]  with  order = stable argsort(y_pred)

Strategy (8 NeuronCores, data parallel over batch):
  - Each core gets 4 of the 32 batches -> a [16384, 512] fp32 slab.
  - The channel permutation `order` is computed on the host (y_pred is tiny)
    and baked into the program as "runs" (maximal stretches where
    order[c+1] == order[c]+1).  Random 8-cluster y_pred gives ~450 runs,
    pair-merged into ~230 copy instructions via an extra AP dim.
  - Per core the slab is processed in 8 tiles of [128 part x 16 rows x 512 ch]
    (4 MiB): contiguous DMA load (SP/HWDGE), on-chip gather (DVE strided
    copies), contiguous DMA store (ACT/HWDGE).  Double buffered; the gather
    hides under the ~187us/core DMA roofline.

Raw Bass (not Tile): the Tile framework inlines semaphore waits into
instructions, and the hardware allows only 1 inline wait on a DMA and 2 on a
TensorCopy -- the slot-reuse wait sets here exceed that.  With explicit
standalone wait_ge instructions there is no such limit.
"""

import functools
import os
from contextlib import ExitStack

import numpy as np

import concourse.bass as bass
import concourse.mybir as mybir
from concourse.ap import AP
from concourse import bass_utils

N_CORES = 8
B, L, C = 32, 4096, 512
B_PER_CORE = B // N_CORES          # 4
ROWS = B_PER_CORE * L              # 16384 rows per core
P = 128                            # SBUF partitions
R_LO = int(os.environ.get("K_RLO", "24"))  # rows per partition per tile
# "quad" = parallelogram-merged length-1 runs (4-dim APs, two pair dims).
# NOTE: quads are only stable with the small pair dim innermost (see
# _job_ap); with the rows dim innermost they intermittently wedged the
# device.  "pair" (max one pair dim) kept as a fallback knob.
K_PLAN = os.environ.get("K_PLAN", "quad")  # quad | pair
TILE_ROWS = P * R_LO
F32 = mybir.dt.float32


def _runs_from_order(order, c=C):
    """Maximal runs (dst_start, src_start, length) with order[d+i] == s+i."""
    runs = []
    start = 0
    for i in range(1, c + 1):
        if i == c or order[i] != order[i - 1] + 1:
            runs.append((start, int(order[start]), i - start))
            start = i
    return runs


def _plan_jobs(runs):
    """Merge runs into copy jobs, minimizing instruction count.

    A job is (d0, s0, length, extra) where extra is a tuple of up to two
    (dst_step, src_step) dims of count 2.  The AP is
    [partition] + [(step, 2) per extra] + [rows] + ([1, length] if length>1).
    The engine AP limit is 4 dims total, so:
      - length==1 runs: up to 2 extra dims -> merge FOUR runs per
        instruction when they form a parallelogram in (dst, src) space
        (two run-pairs with the same difference vector).
      - length>=2 runs: 1 extra dim -> merge pairs of equal-length runs.
    """
    from collections import defaultdict

    jobs = []
    singles = [r for r in runs if r[2] == 1]
    longs = [r for r in runs if r[2] > 1]
    if K_PLAN == "pair":
        singles, longs = [], runs

    # --- quad-match length-1 runs (parallelogram matching) ---
    n = len(singles)
    buckets = defaultdict(list)
    for i in range(n):
        di, si, _ = singles[i]
        for j in range(i + 1, n):
            dj, sj, _ = singles[j]
            buckets[(dj - di, sj - si)].append((i, j))
    used = [False] * n
    for vec, plist in sorted(buckets.items(), key=lambda kv: -len(kv[1])):
        if len(plist) < 2:
            continue
        chosen, taken = [], set()
        for i, j in plist:
            if used[i] or used[j] or i in taken or j in taken:
                continue
            chosen.append((i, j))
            taken.update((i, j))
        while len(chosen) >= 2:
            i, j = chosen.pop(0)
            k, l = chosen.pop(0)
            for idx in (i, j, k, l):
                used[idx] = True
            d0, s0, _ = singles[i]
            outer = (singles[k][0] - d0, singles[k][1] - s0)
            inner = vec
            jobs.append((d0, s0, 1, (outer, inner)))

    # --- pair leftovers (and length>=2 runs), one extra dim ---
    def pair_up(rs):
        rs = sorted(rs)
        for a in range(0, len(rs) - 1, 2):
            d0, s0, length = rs[a]
            d1, s1, _ = rs[a + 1]
            jobs.append((d0, s0, length, ((d1 - d0, s1 - s0),)))
        if len(rs) % 2:
            d0, s0, length = rs[-1]
            jobs.append((d0, s0, length, ()))

    pair_up([singles[i] for i in range(n) if not used[i]])
    by_len = defaultdict(list)
    for r in longs:
        by_len[r[2]].append(r)
    for _, rs in sorted(by_len.items()):
        pair_up(rs)
    return jobs


def _job_ap(tile_ap, job, use_dst):
    """AP for a job over an SBUF tile viewed as [128, R, C].

    Dim order: [partition][outer pair][rows][inner pair or length].  The
    large-stride rows dim is deliberately NOT innermost when two pair dims
    exist -- quads with the rows dim innermost intermittently wedged the
    device; with a small pair dim innermost (the same shape class as the
    stable length>=2 pair jobs) they behave."""
    d0, s0, length, extra = job
    o0 = d0 if use_dst else s0
    base = tile_ap[:, :, o0:o0 + length]
    dims = [base.ap[0]]
    pair_dims = [[dd if use_dst else ds, 2] for dd, ds in extra]
    if len(pair_dims) == 2:
        assert length == 1
        dims += [pair_dims[0], base.ap[1], pair_dims[1]]
    else:
        dims += pair_dims + [base.ap[1]]
        if length > 1:
            dims.append(base.ap[2])
    if not extra and length > 1:
        return base
    return AP(tensor=base.tensor, offset=base.offset, ap=dims)


def _emit(nc, x_ap, out_ap, order, rows, r_lo=R_LO):
    """Emit the per-core program: rows x 512 slab, gather channels by order."""
    jobs = _plan_jobs(_runs_from_order(order))
    rows_pp = rows // P  # rows per partition overall
    # Small first tile -> short pipeline fill; small last tile -> short
    # drain (tail = last gather + last store happens after the final load).
    if rows_pp > 2 * r_lo:
        mid = rows_pp - 16
        tile_rs = [8] + [r_lo] * (mid // r_lo)
        if mid % r_lo:
            tile_rs.append(mid % r_lo)
        tile_rs.append(8)
    else:
        tile_rs = [r_lo] * (rows_pp // r_lo)
        if rows_pp % r_lo:
            tile_rs.append(rows_pp % r_lo)
    assert sum(tile_rs) == rows_pp
    n_tiles = len(tile_rs)
    free = r_lo * C

    # tile t covers rows [off*P, off*P + P*r), partition p owns r of them
    offs = [sum(tile_rs[:t]) for t in range(n_tiles)]

    def x_tile(t):
        r = tile_rs[t]
        return x_ap[offs[t] * P:(offs[t] + r) * P, :].rearrange(
            "(p r) c -> p (r c)", p=P
        )

    def out_tile(t):
        r = tile_rs[t]
        return out_ap[offs[t] * P:(offs[t] + r) * P, :].rearrange(
            "(p r) c -> p (r c)", p=P
        )

    with ExitStack() as ctx:
        in_bufs = [
            ctx.enter_context(nc.sbuf_tensor(f"t_in{i}", [P, free], F32))
            for i in range(2)
        ]
        out_bufs = [
            ctx.enter_context(nc.sbuf_tensor(f"t_out{i}", [P, free], F32))
            for i in range(2)
        ]
        # Per-parity DMA sems: at most one DMA in flight per sem, so a
        # sem value unambiguously identifies WHICH transfer completed
        # (same-ring DMA completions are not ordered).
        s_load = [
            ctx.enter_context(nc.semaphore(f"s_load{i}")) for i in range(2)
        ]
        s_store = [
            ctx.enter_context(nc.semaphore(f"s_store{i}")) for i in range(2)
        ]
        s_dve = ctx.enter_context(nc.semaphore("s_dve"))
        ctx.enter_context(nc.Block())
        block = nc.cur_block

        @block.sync
        def _(sync):
            for t in range(n_tiles):
                if t >= 2:
                    # in_bufs[t%2] is free once tile t-2's copies retired
                    sync.wait_ge(s_dve, t - 1)
                r = tile_rs[t]
                sync.dma_start(
                    in_bufs[t % 2][:, :r * C], x_tile(t)
                ).then_inc(s_load[t % 2], 16)

        @block.scalar
        def _(scalar):
            for t in range(n_tiles):
                scalar.wait_ge(s_dve, t + 1)  # tile t fully gathered
                r = tile_rs[t]
                scalar.dma_start(
                    out_tile(t), out_bufs[t % 2][:, :r * C]
                ).then_inc(s_store[t % 2], 16)
            # Drain: the program must not end with store transfers still in
            # flight (teardown while a DMA is outstanding intermittently
            # kills the exec unit).  DVE only ever waits stores up to t-2,
            # so explicitly await every store completion here.
            for par in range(2):
                n_par = len([t for t in range(n_tiles) if t % 2 == par])
                if n_par:
                    scalar.wait_ge(s_store[par], 16 * n_par)

        @block.vector
        def _(vector):
            for t in range(n_tiles):
                vector.wait_ge(s_load[t % 2], 16 * (t // 2 + 1))  # load t done
                if t >= 2:
                    # out_bufs[t%2] is free once store t-2 drained
                    vector.wait_ge(s_store[t % 2], 16 * ((t - 2) // 2 + 1))
                r = tile_rs[t]
                src_t = in_bufs[t % 2][:, :r * C].rearrange(
                    "p (r c) -> p r c", c=C
                )
                dst_t = out_bufs[t % 2][:, :r * C].rearrange(
                    "p (r c) -> p r c", c=C
                )
                for i, job in enumerate(jobs):
                    ins = vector.tensor_copy(
                        out=_job_ap(dst_t, job, True),
                        in_=_job_ap(src_t, job, False),
                    )
                    if i == len(jobs) - 1:
                        ins.then_inc(s_dve, 1)


@functools.lru_cache(maxsize=4)
def _build(order_key):
    nc = bass.Bass("TRN2")
    x = nc.dram_tensor("x", [ROWS, C], F32, kind="ExternalInput")
    out = nc.dram_tensor("out", [ROWS, C], F32, kind="ExternalOutput")
    _emit(nc, x[:], out[:], list(order_key), ROWS)
    return nc


def _run(x, y_pred, trace=False, trace_cores=None):
    x = np.ascontiguousarray(np.asarray(x), dtype=np.float32)
    y_pred = np.asarray(y_pred)
    assert x.shape == (B, L, C), x.shape
    order = np.argsort(y_pred, kind="stable")
    nc = _build(tuple(int(v) for v in order))

    shards = [
        np.ascontiguousarray(
            x[i * B_PER_CORE:(i + 1) * B_PER_CORE].reshape(ROWS, C)
        )
        for i in range(N_CORES)
    ]
    in_maps = [{"x": s} for s in shards]
    res = bass_utils.run_bass_kernel_spmd(
        nc,
        in_maps,
        core_ids=list(range(N_CORES)),
        trace=trace,
        trace_cores=trace_cores,
    )
    out = np.concatenate(
        [r["out"].reshape(B_PER_CORE, L, C) for r in res.results], axis=0
    )
    return out, res


def kernel(x, y_pred):
    out, _ = _run(x, y_pred, trace=False)
    return out


# revision 24
# speedup vs baseline: 1.2128x; 1.0940x over previous
"""Channel-permutation (stable bucket sort by cluster id) kernel for TRN2.

out[b, l, c] = x[b, l, order[c]]  with  order = stable argsort(y_pred)

Strategy (8 NeuronCores, data parallel over batch):
  - Each core gets 4 of the 32 batches -> a [16384, 512] fp32 slab.
  - The channel permutation `order` is computed on the host (y_pred is tiny)
    and baked into the program as "runs" (maximal stretches where
    order[c+1] == order[c]+1).  Random 8-cluster y_pred gives ~450 runs,
    merged into ~140 copy instructions: length-1 runs are quad-merged when
    two run-pairs share a (dst,src) difference vector (parallelogram match,
    two extra AP dims), remaining runs pair-merged (one extra AP dim).
  - Per core the slab is processed in tiles of [128 part x R rows x 512 ch]
    with R per tile [8,24,24,24,24,16,8] (small first/last tiles shorten
    pipeline fill/drain): contiguous DMA load (SP/HWDGE), on-chip gather
    (DVE strided copies), contiguous DMA store (ACT/HWDGE).  Double
    buffered; the gather hides under the ~187us/core DMA roofline.
    Measured: ~186-209us HW exec, bit-exact.

Raw Bass (not Tile): the Tile framework inlines semaphore waits into
instructions, and the hardware allows only 1 inline wait on a DMA and 2 on a
TensorCopy -- the slot-reuse wait sets here exceed that.  With explicit
standalone wait_ge instructions there is no such limit.
"""

import functools
import os
from contextlib import ExitStack

import numpy as np

import concourse.bass as bass
import concourse.mybir as mybir
from concourse.ap import AP
from concourse import bass_utils

N_CORES = 8
B, L, C = 32, 4096, 512
B_PER_CORE = B // N_CORES          # 4
ROWS = B_PER_CORE * L              # 16384 rows per core
P = 128                            # SBUF partitions
R_LO = int(os.environ.get("K_RLO", "24"))  # rows per partition per tile
# "quad" = parallelogram-merged length-1 runs (4-dim APs, two pair dims).
# NOTE: quads are only stable with the small pair dim innermost (see
# _job_ap); with the rows dim innermost they intermittently wedged the
# device.  "pair" (max one pair dim) kept as a fallback knob.
K_PLAN = os.environ.get("K_PLAN", "quad")  # quad | pair
TILE_ROWS = P * R_LO
F32 = mybir.dt.float32


def _runs_from_order(order, c=C):
    """Maximal runs (dst_start, src_start, length) with order[d+i] == s+i."""
    runs = []
    start = 0
    for i in range(1, c + 1):
        if i == c or order[i] != order[i - 1] + 1:
            runs.append((start, int(order[start]), i - start))
            start = i
    return runs


def _plan_jobs(runs):
    """Merge runs into copy jobs, minimizing instruction count.

    A job is (d0, s0, length, extra) where extra is a tuple of up to two
    (dst_step, src_step) dims of count 2.  The AP is
    [partition] + [(step, 2) per extra] + [rows] + ([1, length] if length>1).
    The engine AP limit is 4 dims total, so:
      - length==1 runs: up to 2 extra dims -> merge FOUR runs per
        instruction when they form a parallelogram in (dst, src) space
        (two run-pairs with the same difference vector).
      - length>=2 runs: 1 extra dim -> merge pairs of equal-length runs.
    """
    from collections import defaultdict

    jobs = []
    singles = [r for r in runs if r[2] == 1]
    longs = [r for r in runs if r[2] > 1]
    if K_PLAN == "pair":
        singles, longs = [], runs

    # --- quad-match length-1 runs (parallelogram matching) ---
    n = len(singles)
    buckets = defaultdict(list)
    for i in range(n):
        di, si, _ = singles[i]
        for j in range(i + 1, n):
            dj, sj, _ = singles[j]
            buckets[(dj - di, sj - si)].append((i, j))
    used = [False] * n
    for vec, plist in sorted(buckets.items(), key=lambda kv: -len(kv[1])):
        if len(plist) < 2:
            continue
        chosen, taken = [], set()
        for i, j in plist:
            if used[i] or used[j] or i in taken or j in taken:
                continue
            chosen.append((i, j))
            taken.update((i, j))
        while len(chosen) >= 2:
            i, j = chosen.pop(0)
            k, l = chosen.pop(0)
            for idx in (i, j, k, l):
                used[idx] = True
            d0, s0, _ = singles[i]
            outer = (singles[k][0] - d0, singles[k][1] - s0)
            inner = vec
            jobs.append((d0, s0, 1, (outer, inner)))

    # --- pair leftovers (and length>=2 runs), one extra dim ---
    def pair_up(rs):
        rs = sorted(rs)
        for a in range(0, len(rs) - 1, 2):
            d0, s0, length = rs[a]
            d1, s1, _ = rs[a + 1]
            jobs.append((d0, s0, length, ((d1 - d0, s1 - s0),)))
        if len(rs) % 2:
            d0, s0, length = rs[-1]
            jobs.append((d0, s0, length, ()))

    pair_up([singles[i] for i in range(n) if not used[i]])
    by_len = defaultdict(list)
    for r in longs:
        by_len[r[2]].append(r)
    for _, rs in sorted(by_len.items()):
        pair_up(rs)
    return jobs


def _job_ap(tile_ap, job, use_dst):
    """AP for a job over an SBUF tile viewed as [128, R, C].

    Dim order: [partition][outer pair][rows][inner pair or length].  The
    large-stride rows dim is deliberately NOT innermost when two pair dims
    exist -- quads with the rows dim innermost intermittently wedged the
    device; with a small pair dim innermost (the same shape class as the
    stable length>=2 pair jobs) they behave."""
    d0, s0, length, extra = job
    o0 = d0 if use_dst else s0
    base = tile_ap[:, :, o0:o0 + length]
    dims = [base.ap[0]]
    pair_dims = [[dd if use_dst else ds, 2] for dd, ds in extra]
    if len(pair_dims) == 2:
        assert length == 1
        dims += [pair_dims[0], base.ap[1], pair_dims[1]]
    else:
        dims += pair_dims + [base.ap[1]]
        if length > 1:
            dims.append(base.ap[2])
    if not extra and length > 1:
        return base
    return AP(tensor=base.tensor, offset=base.offset, ap=dims)


def _split_runs_at(runs, cut):
    """Split runs crossing dst channel `cut`; partition into (lo, hi)."""
    lo, hi = [], []
    for d, s, l in runs:
        if d < cut < d + l:
            lo.append((d, s, cut - d))
            hi.append((cut, s + (cut - d), l - (cut - d)))
        elif d < cut:
            lo.append((d, s, l))
        else:
            hi.append((d, s, l))
    return lo, hi


def _emit(nc, x_ap, out_ap, order, rows, r_lo=R_LO):
    """Emit the per-core program: rows x 512 slab, gather channels by order."""
    runs = _runs_from_order(order)
    jobs = _plan_jobs(runs)
    # Last tile: gather+store in two channel halves so the final store
    # overlaps the final gather (shortens the kernel tail).
    runs_lo, runs_hi = _split_runs_at(runs, C // 2)
    last_groups = [
        (0, C // 2, _plan_jobs(runs_lo)),
        (C // 2, C, _plan_jobs(runs_hi)),
    ]
    rows_pp = rows // P  # rows per partition overall
    # Small first tile -> short pipeline fill; small last tile -> short
    # drain (tail = last gather + last store happens after the final load).
    if rows_pp > 2 * r_lo:
        mid = rows_pp - 16
        tile_rs = [8] + [r_lo] * (mid // r_lo)
        if mid % r_lo:
            tile_rs.append(mid % r_lo)
        tile_rs.append(8)
    else:
        tile_rs = [r_lo] * (rows_pp // r_lo)
        if rows_pp % r_lo:
            tile_rs.append(rows_pp % r_lo)
    assert sum(tile_rs) == rows_pp
    n_tiles = len(tile_rs)
    free = r_lo * C

    # tile t covers rows [off*P, off*P + P*r), partition p owns r of them
    offs = [sum(tile_rs[:t]) for t in range(n_tiles)]

    def x_tile(t):
        r = tile_rs[t]
        return x_ap[offs[t] * P:(offs[t] + r) * P, :].rearrange(
            "(p r) c -> p (r c)", p=P
        )

    def out_tile(t):
        r = tile_rs[t]
        return out_ap[offs[t] * P:(offs[t] + r) * P, :].rearrange(
            "(p r) c -> p (r c)", p=P
        )

    with ExitStack() as ctx:
        in_bufs = [
            ctx.enter_context(nc.sbuf_tensor(f"t_in{i}", [P, free], F32))
            for i in range(2)
        ]
        out_bufs = [
            ctx.enter_context(nc.sbuf_tensor(f"t_out{i}", [P, free], F32))
            for i in range(2)
        ]
        # Per-parity DMA sems: at most one DMA in flight per sem, so a
        # sem value unambiguously identifies WHICH transfer completed
        # (same-ring DMA completions are not ordered).
        s_load = [
            ctx.enter_context(nc.semaphore(f"s_load{i}")) for i in range(2)
        ]
        s_store = [
            ctx.enter_context(nc.semaphore(f"s_store{i}")) for i in range(2)
        ]
        s_dve = ctx.enter_context(nc.semaphore("s_dve"))
        # Keep the full exit drain (incl. GPSIMD): no_gpsimd_drain=True would
        # trim ~3us of kernel tail but runs following it show ~1-2min device
        # recovery stalls (teardown left unclean) -- not worth it.
        ctx.enter_context(nc.Block())
        block = nc.cur_block

        @block.sync
        def _(sync):
            for t in range(n_tiles):
                if t >= 2:
                    # in_bufs[t%2] is free once tile t-2's copies retired
                    sync.wait_ge(s_dve, t - 1)
                r = tile_rs[t]
                sync.dma_start(
                    in_bufs[t % 2][:, :r * C], x_tile(t)
                ).then_inc(s_load[t % 2], 16)

        last = n_tiles - 1
        store_counts = [0, 0]  # s_store increments (x16) per parity

        @block.scalar
        def _(scalar):
            for t in range(n_tiles):
                r = tile_rs[t]
                if t < last:
                    scalar.wait_ge(s_dve, t + 1)  # tile t fully gathered
                    scalar.dma_start(
                        out_tile(t), out_bufs[t % 2][:, :r * C]
                    ).then_inc(s_store[t % 2], 16)
                    store_counts[t % 2] += 1
                else:
                    sb3 = out_bufs[t % 2][:, :r * C].rearrange(
                        "p (r c) -> p r c", c=C
                    )
                    dr3 = out_ap[offs[t] * P:(offs[t] + r) * P, :].rearrange(
                        "(p r) c -> p r c", p=P
                    )
                    for gi, (a, b, _) in enumerate(last_groups):
                        scalar.wait_ge(s_dve, t + 1 + gi)
                        scalar.dma_start(
                            dr3[:, :, a:b], sb3[:, :, a:b]
                        ).then_inc(s_store[t % 2], 16)
                        store_counts[t % 2] += 1
            # Drain: the program must not end with store transfers still in
            # flight (teardown while a DMA is outstanding intermittently
            # kills the exec unit).  DVE only ever waits stores up to t-2,
            # so explicitly await every store completion here.
            for par in range(2):
                if store_counts[par]:
                    scalar.wait_ge(s_store[par], 16 * store_counts[par])

        @block.vector
        def _(vector):
            for t in range(n_tiles):
                vector.wait_ge(s_load[t % 2], 16 * (t // 2 + 1))  # load t done
                if t >= 2:
                    # out_bufs[t%2] is free once store t-2 drained
                    vector.wait_ge(s_store[t % 2], 16 * ((t - 2) // 2 + 1))
                r = tile_rs[t]
                src_t = in_bufs[t % 2][:, :r * C].rearrange(
                    "p (r c) -> p r c", c=C
                )
                dst_t = out_bufs[t % 2][:, :r * C].rearrange(
                    "p (r c) -> p r c", c=C
                )
                groups = (
                    [jobs] if t < last else [g for _, _, g in last_groups]
                )
                for grp in groups:
                    for i, job in enumerate(grp):
                        ins = vector.tensor_copy(
                            out=_job_ap(dst_t, job, True),
                            in_=_job_ap(src_t, job, False),
                        )
                        if i == len(grp) - 1:
                            ins.then_inc(s_dve, 1)


@functools.lru_cache(maxsize=4)
def _build(order_key):
    nc = bass.Bass("TRN2")
    x = nc.dram_tensor("x", [ROWS, C], F32, kind="ExternalInput")
    out = nc.dram_tensor("out", [ROWS, C], F32, kind="ExternalOutput")
    _emit(nc, x[:], out[:], list(order_key), ROWS)
    return nc


def _run(x, y_pred, trace=False, trace_cores=None):
    x = np.ascontiguousarray(np.asarray(x), dtype=np.float32)
    y_pred = np.asarray(y_pred)
    assert x.shape == (B, L, C), x.shape
    order = np.argsort(y_pred, kind="stable")
    nc = _build(tuple(int(v) for v in order))

    shards = [
        np.ascontiguousarray(
            x[i * B_PER_CORE:(i + 1) * B_PER_CORE].reshape(ROWS, C)
        )
        for i in range(N_CORES)
    ]
    in_maps = [{"x": s} for s in shards]
    res = bass_utils.run_bass_kernel_spmd(
        nc,
        in_maps,
        core_ids=list(range(N_CORES)),
        trace=trace,
        trace_cores=trace_cores,
    )
    out = np.concatenate(
        [r["out"].reshape(B_PER_CORE, L, C) for r in res.results], axis=0
    )
    return out, res


def kernel(x, y_pred):
    out, _ = _run(x, y_pred, trace=False)
    return out
